# revision 13
# baseline (speedup 1.0000x reference)
"""AdaptiveCornerLoss on 8 TRN2 NeuronCores — batch-parallel Bass/Tile kernel.

Shapes (hardcoded): B=64, N=16384, C=6, M=128 corners. 8 cores, 8 samples/core.

Math:
  focal    = u^2 * ce  with  y=(1-2t)*x, ce=softplus(y)=ln(1+e^y),
             u=sigmoid(y)  =>  u^2 = exp(-2*ln(1+e^{-y}))
  d2(n,m)  = |p|^2 + |c|^2 - 2 p.c   (augmented fp16 matmul; per-point feature
             rows [px,py,pz,|p|^2,1] vs corner rows [-2cx,-2cy,-2cz,1,|c|^2+pen])
  w        = exp(-10*sqrt(max(min_m d2, 1e-12))) via exp/ln only (one ACT set)

Layout/engine tricks:
  * Valid corners host-compacted; kernel built for Mk = roundup(max valid, 32)
    (96 on the graded data). Padding corners carry |c|^2+pen -> never win.
  * Per chunk the Mk corner columns become [A(NP) | E(NP) | U(Mk-2NP)]:
    corners 0..2NP-1 are paired; A = d2 vs pair-evens, E = d2_even - d2_odd
    (difference features are linear -> same matmul), U = d2 vs unpaired.
    ACT computes relu([E|U]) out of PSUM in ONE op (relu(d2)=d2, clamping
    roundoff negatives); DVE drains only A: pair-min = A - relu(E), then a
    bf16 2x-mode min tree over [pair-mins | U-mins]. PSUM drain splits
    NP:(NP+U) = 1:2 between DVE and ACT.
  * CPG=4 chunks share one LDWEIGHTS; their zero-padded rhs variants are
    adjacent so ONE matmul per group computes 4 chunks (256 LDW+MM total).
  * 2-bank PSUM tiles, 4-deep pipeline; grid DMAs issued first (PE ramps
    early); focal emitted after sample 0; epilogue split in halves.
Outputs per core: per-partition partial sums [128,4]; host reduces.
"""

import sys

sys.path.insert(0, "/opt/trn_rl_repo")
sys.path.insert(0, "/root/problem")

import numpy as np

import concourse.bass as bass
import concourse.mybir as mybir
from concourse import tile
from concourse.bass_utils import run_bass_kernel_spmd
from waitsplit import split_waits

NCORES = 8
B, N, M = 64, 16384, 128
S = B // NCORES          # samples per core
K = 5                    # feature rows per chunk
CPG = 4                  # chunks sharing one LDWEIGHTS group / one matmul
CH = N // 128            # 128-point chunks per sample (128)
GRP = CH // CPG          # groups per sample (32)
COLS = S * CH            # minsq/logit columns per core (1024)
PEN = 100.0

F = mybir.ActivationFunctionType
OP = mybir.AluOpType
DT = mybir.dt

_CACHE = {}


def _split(Mk):
    NP = Mk // 3 + (-(Mk // 3) % 16)   # pairs, rounded up to 16
    U = Mk - 2 * NP                    # unpaired singles
    assert NP % 16 == 0 and U % 16 == 0 and U >= 16, (Mk, NP, U)
    return NP, U


def build_nc(Mk):
    NP, U = _split(Mk)
    Uh = U // 2
    nc = bass.Bass()
    lhsT = nc.declare_dram_parameter(
        "lhsT", [S, K * CPG, N // CPG], DT.float16, isOutput=False
    )
    rhs = nc.declare_dram_parameter(
        "rhs", [K * CPG, S * CPG * Mk], DT.float16, isOutput=False
    )
    lg = nc.declare_dram_parameter("lg", [128, COLS], DT.float32, isOutput=False)
    tg = nc.declare_dram_parameter("tg", [128, COLS], DT.float32, isOutput=False)
    out = nc.declare_dram_parameter("out", [128, 4], DT.float32, isOutput=True)

    with tile.TileContext(nc) as tc:
        with (
            tc.tile_pool(name="persist", bufs=1) as pp,
            tc.tile_pool(name="stream", bufs=2) as wp,
            tc.tile_pool(name="relupool", bufs=4) as rp,
            tc.tile_pool(name="psum", bufs=4, space="PSUM") as psp,
        ):
            # --- grid inputs first so the PE pipeline ramps immediately
            rt = pp.tile([K * CPG, S * CPG * Mk], DT.float16)
            nc.sync.dma_start(out=rt[:], in_=rhs[:])
            lts = [wp.tile([K * CPG, N // CPG], DT.float16, tag="lhsT",
                           name=f"lt{i}") for i in range(2)]
            nc.sync.dma_start(out=lts[0][:], in_=lhsT[0])
            lgt = pp.tile([128, COLS], DT.float32)
            nc.sync.dma_start(out=lgt[:], in_=lg[:])
            tgt = pp.tile([128, COLS], DT.float32)
            nc.sync.dma_start(out=tgt[:], in_=tg[:])

            sums = pp.tile([128, 4], DT.float32)
            minsq = pp.tile([128, COLS], DT.float32)
            y = pp.tile([128, COLS], DT.float32)
            ce = pp.tile([128, COLS], DT.float32)
            u2 = pp.tile([128, COLS], DT.float32)
            fo = pp.tile([128, COLS], DT.float32)

            def emit_grid(s):
                lt = lts[s % 2]
                if s + 1 < S:
                    nxt = lts[(s + 1) % 2]
                    nc.sync.dma_start(out=nxt[:], in_=lhsT[s + 1])
                for tp in range(4):  # 4 tree batches of 32 chunks
                    pmb = wp.tile([128, 32 * NP], DT.bfloat16, tag="pmb")
                    umb = wp.tile([128, 32 * Uh], DT.bfloat16, tag="umb")
                    for half in range(4):
                        t = tp * 4 + half
                        pt = psp.tile([128, 1024], DT.float32)  # 2 banks, 8 chunks
                        for bk in range(2):
                            g = t * 2 + bk
                            nc.tensor.matmul(
                                out=pt[:, 512 * bk: 512 * bk + CPG * Mk],
                                lhsT=lt[:, g * 128:(g + 1) * 128],
                                rhs=rt[:, s * CPG * Mk:(s + 1) * CPG * Mk],
                                start=True, stop=True,
                            )
                        grid = pt[:].rearrange("p (b r) -> p b r", r=512)
                        grid = grid[:, :, 0: CPG * Mk]
                        grid = grid.rearrange("p b (v m) -> p b v m", m=Mk)
                        # ACT: rl = relu([E|U]) straight out of PSUM (one op)
                        rl = rp.tile([128, 8 * (NP + U)], DT.float32, tag="relu")
                        rlv4 = rl[:].rearrange(
                            "p (b v m) -> p b v m", v=CPG, m=NP + U
                        )
                        nc.scalar.activation(rlv4, grid[:, :, :, NP:Mk], F.Relu)
                        rlv = rl[:].rearrange("p (c m) -> p c m", m=NP + U)
                        # DVE: pair-min = A - relu(E)  (bf16 out)
                        nc.vector.tensor_tensor(
                            out=pmb[:, half * 8 * NP:(half + 1) * 8 * NP]
                                .rearrange("p (c m) -> p c m", m=NP),
                            in0=grid[:, :, :, 0:NP],
                            in1=rlv[:, :, 0:NP],
                            op=OP.subtract,
                        )
                        # DVE: first U level: min of relu'd singles (fp32->bf16)
                        nc.vector.tensor_tensor(
                            out=umb[:, half * 8 * Uh:(half + 1) * 8 * Uh]
                                .rearrange("p (c m) -> p c m", m=Uh),
                            in0=rlv[:, :, NP:NP + Uh],
                            in1=rlv[:, :, NP + Uh:NP + U],
                            op=OP.min,
                        )
                    # bf16 2x min tree over the 32-chunk batch
                    pmv = pmb[:].rearrange("p (c m) -> p c m", m=NP)
                    m2 = wp.tile([128, 32 * (NP // 2)], DT.bfloat16, tag="m2")
                    nc.vector.tensor_tensor(
                        out=m2[:].rearrange("p (c m) -> p c m", m=NP // 2),
                        in0=pmv[:, :, 0:NP // 2],
                        in1=pmv[:, :, NP // 2:NP],
                        op=OP.min,
                    )
                    # fold down to Uh width then merge with U-mins
                    cur = m2
                    width = NP // 2
                    while width > Uh:
                        nxt = wp.tile([128, 32 * (width // 2)], DT.bfloat16,
                                      tag=f"mt{width}")
                        nc.vector.tensor_tensor(
                            out=nxt[:].rearrange("p (c m) -> p c m", m=width // 2),
                            in0=cur[:].rearrange("p (c m) -> p c m", m=width)
                                [:, :, 0:width // 2],
                            in1=cur[:].rearrange("p (c m) -> p c m", m=width)
                                [:, :, width // 2:width],
                            op=OP.min,
                        )
                        cur = nxt
                        width //= 2
                    assert width == Uh
                    mg = wp.tile([128, 32 * width], DT.bfloat16, tag="mg")
                    nc.vector.tensor_tensor(
                        out=mg[:].rearrange("p (c m) -> p c m", m=width),
                        in0=cur[:].rearrange("p (c m) -> p c m", m=width),
                        in1=umb[:].rearrange("p (c m) -> p c m", m=Uh),
                        op=OP.min,
                    )
                    # one more halving, then reduce
                    mh = wp.tile([128, 32 * (width // 2)], DT.bfloat16, tag="mh")
                    nc.vector.tensor_tensor(
                        out=mh[:].rearrange("p (c m) -> p c m", m=width // 2),
                        in0=mg[:].rearrange("p (c m) -> p c m", m=width)
                            [:, :, 0:width // 2],
                        in1=mg[:].rearrange("p (c m) -> p c m", m=width)
                            [:, :, width // 2:width],
                        op=OP.min,
                    )
                    c0 = s * CH + tp * 32
                    nc.vector.tensor_reduce(
                        out=minsq[:, c0:c0 + 32],
                        in_=mh[:].rearrange("p (c m) -> p c m", m=width // 2),
                        axis=mybir.AxisListType.X,
                        op=OP.min,
                    )

            def emit_focal(h):
                c0, c1 = h * (COLS // 2), (h + 1) * (COLS // 2)
                nc.vector.tensor_scalar(
                    out=y[:, c0:c1], in0=tgt[:, c0:c1], scalar1=-2.0, scalar2=1.0,
                    op0=OP.mult, op1=OP.add,
                )
                nc.gpsimd.tensor_tensor(
                    out=y[:, c0:c1], in0=y[:, c0:c1], in1=lgt[:, c0:c1], op=OP.mult
                )
                nc.scalar.activation(ce[:, c0:c1], y[:, c0:c1], F.Exp)
                nc.scalar.activation(ce[:, c0:c1], ce[:, c0:c1], F.Ln, bias=1.0)
                nc.scalar.activation(u2[:, c0:c1], y[:, c0:c1], F.Exp, scale=-1.0)
                nc.scalar.activation(u2[:, c0:c1], u2[:, c0:c1], F.Ln, bias=1.0)
                nc.scalar.activation(u2[:, c0:c1], u2[:, c0:c1], F.Exp, scale=-2.0)
                nc.gpsimd.tensor_tensor(
                    out=fo[:, c0:c1], in0=ce[:, c0:c1], in1=u2[:, c0:c1], op=OP.mult
                )
                nc.vector.tensor_reduce(
                    out=sums[:, h:h + 1], in_=fo[:, c0:c1],
                    axis=mybir.AxisListType.X, op=OP.add,
                )

            def emit_epilogue(h):
                c0, c1 = h * (COLS // 2), (h + 1) * (COLS // 2)
                ms = minsq[:, c0:c1]
                nc.vector.tensor_scalar_max(out=ms, in0=ms, scalar1=1e-12)
                nc.scalar.activation(ms, ms, F.Ln)
                nc.scalar.activation(ms, ms, F.Exp, scale=0.5)
                nc.scalar.activation(ms, ms, F.Exp, scale=-10.0)
                nc.gpsimd.tensor_tensor(out=y[:, c0:c1], in0=fo[:, c0:c1],
                                        in1=ms, op=OP.mult)
                nc.vector.tensor_reduce(
                    out=sums[:, 2 + h:3 + h], in_=y[:, c0:c1],
                    axis=mybir.AxisListType.X, op=OP.add,
                )

            emit_grid(0)
            emit_focal(0)
            emit_focal(1)
            for s in range(1, S):
                emit_grid(s)
                if s == S - 2:
                    emit_epilogue(0)   # samples 0..3 columns are final
            emit_epilogue(1)
            nc.sync.dma_start(out=out[:], in_=sums[:])

    split_waits(nc)
    return nc


def pack_inputs(inputs, targets, point_coords, corner_coords):
    """Host-side shard + layout packing. Returns (in_maps, Mk)."""
    x = np.asarray(inputs, np.float32)
    t = np.asarray(targets, np.float32)
    pc = np.asarray(point_coords, np.float32)
    cc = np.asarray(corner_coords, np.float32)

    pts = pc[..., :3]
    q = (pts * pts).sum(-1)
    feats = np.empty((B, K, N), np.float32)
    feats[:, 0] = pts[..., 0]
    feats[:, 1] = pts[..., 1]
    feats[:, 2] = pts[..., 2]
    feats[:, 3] = q
    feats[:, 4] = 1.0
    fg = feats.reshape(B, K, GRP, CPG, 128).transpose(0, 3, 1, 2, 4)
    lhsT = fg.reshape(B, K * CPG, N // CPG).astype(np.float16)

    valid = cc[..., 0] > -1.0
    nv = valid.sum(-1)
    maxv = int(nv.max()) if nv.max() > 0 else 1
    Mk = min(M, ((maxv + 31) // 32) * 32)
    NP, U = _split(Mk)
    cfeat = np.zeros((B, K, Mk), np.float32)
    cfeat[:, 4] = PEN
    for b in range(B):
        v = cc[b][valid[b]]
        n = v.shape[0]
        cfeat[b, 0, :n] = -2.0 * v[:, 0]
        cfeat[b, 1, :n] = -2.0 * v[:, 1]
        cfeat[b, 2, :n] = -2.0 * v[:, 2]
        cfeat[b, 3, :n] = 1.0
        cfeat[b, 4, :n] = (v * v).sum(-1)
    # block per chunk: [A(NP even-of-pair) | E(NP diffs) | U(unpaired)]
    fA = cfeat[:, :, 0:2 * NP:2]
    fB = cfeat[:, :, 1:2 * NP:2]
    fE = fA - fB
    fU = cfeat[:, :, 2 * NP:Mk]
    blk = np.concatenate([fA, fE, fU], axis=2)      # width Mk
    assert blk.shape[2] == Mk
    rhs = np.zeros((B, CPG, K * CPG, Mk), np.float32)
    for v in range(CPG):
        rhs[:, v, v * K:(v + 1) * K, :] = blk
    rhs = rhs.astype(np.float16)

    in_maps = []
    for c in range(NCORES):
        sl = slice(c * S, (c + 1) * S)
        lgp = x[sl].reshape(S, CH, 128).transpose(2, 0, 1).reshape(128, COLS).copy()
        tgp = t[sl].reshape(S, CH, 128).transpose(2, 0, 1).reshape(128, COLS).copy()
        rhp = rhs[sl].transpose(2, 0, 1, 3).reshape(K * CPG, S * CPG * Mk).copy()
        in_maps.append({
            "lhsT": np.ascontiguousarray(lhsT[sl]),
            "rhs": rhp,
            "lg": lgp,
            "tg": tgp,
        })
    return in_maps, Mk


def _finalize(results):
    s1 = 0.0
    s2 = 0.0
    for r in results:
        o = np.asarray(r["out"], np.float64)
        s1 += o[:, 0].sum() + o[:, 1].sum()
        s2 += o[:, 2].sum() + o[:, 3].sum()
    bn = float(B * N)
    focal = s1 / bn
    distance = (s1 + 2.0 * s2) / bn
    total = focal + distance
    return (np.float32(total), np.float32(focal), np.float32(distance))


def kernel(inputs, targets, point_coords, corner_coords):
    in_maps, Mk = pack_inputs(inputs, targets, point_coords, corner_coords)
    if Mk not in _CACHE:
        _CACHE[Mk] = build_nc(Mk)
    nc = _CACHE[Mk]
    res = run_bass_kernel_spmd(nc, in_maps, core_ids=list(range(NCORES)))
    return _finalize(res.results)


if __name__ == "__main__":
    rng = np.random.default_rng(0)
    ins = {
        "inputs": rng.standard_normal((B, N), dtype=np.float32),
        "targets": (rng.random((B, N)) < 0.05).astype(np.float32),
        "point_coords": rng.random((B, N, 6), dtype=np.float32),
        "corner_coords": rng.random((B, 128, 3), dtype=np.float32),
    }
    print(kernel(**ins))


# revision 18
# speedup vs baseline: 1.2633x; 1.2633x over previous
"""AdaptiveCornerLoss on 8 TRN2 NeuronCores — batch-parallel Bass/Tile kernel.

Shapes (hardcoded): B=64, N=16384, C=6, M=128 corners. 8 cores, 8 samples/core.

Math:
  focal    = u^2 * ce  with  y=(1-2t)*x, ce=softplus(y)=ln(1+e^y),
             u=sigmoid(y)  =>  u^2 = exp(-2*ln(1+e^{-y}))
  d2(n,m)  = |p|^2 + |c|^2 - 2 p.c   (augmented fp16 matmul; per-point feature
             rows [px,py,pz,|p|^2,1] vs corner rows [-2cx,-2cy,-2cz,1,|c|^2+pen])
  w        = exp(-10*sqrt(max(min_m d2, 1e-12))) via exp/ln only (one ACT set)

Layout/engine tricks:
  * Valid corners host-compacted per sample; kernel built for Mk =
    roundup(max valid, 32) corners (96 on the graded data). Padding corners
    carry |c|^2+pen so they never win the min.
  * Pairwise-min offload: corners paired (2j, 2j+1). PE emits, per chunk,
    A = d2 vs even corners [Mh cols] and E = d2_even - d2_odd [Mh cols]
    (difference features are linear -> one matmul). ACT computes R=relu(E)
    straight out of PSUM; DVE computes pair-min = A - R (bf16) and a 2x-mode
    bf16 min tree. This splits the PSUM drain evenly between ACT and DVE and
    halves the DVE tree input.
  * CPG=4 chunks share one LDWEIGHTS (stationary K=20 stacks 4 chunks'
    features; the 4 zero-padded rhs variants are adjacent so ONE matmul per
    group computes all 4 chunks: 256 LDW+MM pairs total, not 1024).
  * PSUM tiles span 4 banks; each bank = one group's [128, 4*Mk] grid.
Outputs per core: per-partition partial sums [128,2] of (focal, focal*w);
host reduces and forms (total, focal_loss, distance_loss).
"""

import sys

sys.path.insert(0, "/opt/trn_rl_repo")
sys.path.insert(0, "/root/problem")

import numpy as np

import concourse.bass as bass
import concourse.mybir as mybir
from concourse import tile
from concourse.bass_utils import run_bass_kernel_spmd
from waitsplit import split_waits

NCORES = 8
B, N, M = 64, 16384, 128
S = B // NCORES          # samples per core
K = 5                    # feature rows per chunk
CPG = 4                  # chunks sharing one LDWEIGHTS group / one matmul
CH = N // 128            # 128-point chunks per sample (128)
GRP = CH // CPG          # groups per sample (32)
TPS = 8                  # psum tiles per sample (4 groups = 16 chunks each)
COLS = S * CH            # minsq/logit columns per core (1024)
PEN = 100.0

F = mybir.ActivationFunctionType
OP = mybir.AluOpType
DT = mybir.dt

_CACHE = {}


def build_nc(Mk):
    Mh = Mk // 2
    nc = bass.Bass()
    lhsT = nc.declare_dram_parameter(
        "lhsT", [S, K * CPG, N // CPG], DT.float16, isOutput=False
    )
    rhs = nc.declare_dram_parameter(
        "rhs", [K * CPG, S * CPG * Mk], DT.float16, isOutput=False
    )
    lg = nc.declare_dram_parameter("lg", [128, COLS], DT.float32, isOutput=False)
    tg = nc.declare_dram_parameter("tg", [128, COLS], DT.float32, isOutput=False)
    out = nc.declare_dram_parameter("out", [128, 4], DT.float32, isOutput=True)

    # bf16 min-tree levels: Mh -> ... -> wlast (tensor_reduce finishes)
    levels = []
    w = Mh
    while w % 2 == 0 and w > 3:
        w //= 2
        levels.append(w)

    with tile.TileContext(nc) as tc:
        with (
            tc.tile_pool(name="persist", bufs=1) as pp,
            tc.tile_pool(name="stream", bufs=2) as wp,
            tc.tile_pool(name="relupool", bufs=4) as rp,
            tc.tile_pool(name="psum", bufs=4, space="PSUM") as psp,
        ):
            # --- grid inputs first so the PE pipeline ramps immediately
            rt = pp.tile([K * CPG, S * CPG * Mk], DT.float16)
            nc.sync.dma_start(out=rt[:], in_=rhs[:])
            lts = [wp.tile([K * CPG, N // CPG], DT.float16, tag="lhsT",
                           name=f"lt{i}") for i in range(2)]
            nc.sync.dma_start(out=lts[0][:], in_=lhsT[0])
            lgt = pp.tile([128, COLS], DT.float32)
            nc.sync.dma_start(out=lgt[:], in_=lg[:])
            tgt = pp.tile([128, COLS], DT.float32)
            nc.sync.dma_start(out=tgt[:], in_=tg[:])

            sums = pp.tile([128, 4], DT.float32)
            minsq = pp.tile([128, COLS], DT.float32)
            y = pp.tile([128, COLS], DT.float32)
            ce = pp.tile([128, COLS], DT.float32)
            u2 = pp.tile([128, COLS], DT.float32)
            fo = pp.tile([128, COLS], DT.float32)

            def emit_grid(s):
                lt = lts[s % 2]
                if s + 1 < S:
                    nc.sync.dma_start(out=lts[(s + 1) % 2][:], in_=lhsT[s + 1])
                for tp in range(4):  # 4 tree batches of 32 chunks per sample
                    trb = wp.tile([128, 32 * Mh], DT.bfloat16, tag="tree0")
                    for half in range(4):
                        t = tp * 4 + half
                        pt = psp.tile([128, 1024], DT.float32)  # 2 banks, 8 chunks
                        for bk in range(2):
                            g = t * 2 + bk
                            for ct in range(2):  # concurrent 64-point col tiles
                                nc.tensor.matmul(
                                    out=pt[64 * ct: 64 * (ct + 1),
                                           512 * bk: 512 * bk + CPG * Mk],
                                    lhsT=lt[:, g * 128 + 64 * ct:
                                            g * 128 + 64 * (ct + 1)],
                                    rhs=rt[:, s * CPG * Mk:(s + 1) * CPG * Mk],
                                    start=True, stop=True,
                                    tile_position=(0, 64 * ct),
                                )
                        grid = pt[:].rearrange("p (b r) -> p b r", r=512)
                        grid = grid[:, :, 0: CPG * Mk]
                        grid = grid.rearrange("p b (v m) -> p b v m", m=Mk)
                        # R = relu(E) out of PSUM (ACT), pair-min = A - R (DVE)
                        rl = rp.tile([128, 8 * Mh], DT.float32, tag="relu")
                        nc.scalar.activation(rl[:], grid[:, :, :, Mh:Mk], F.Relu)
                        nc.vector.tensor_tensor(
                            out=trb[:, half * 8 * Mh:(half + 1) * 8 * Mh],
                            in0=grid[:, :, :, 0:Mh],
                            in1=rl[:].rearrange("p (b v m) -> p b v m",
                                                v=CPG, m=Mh),
                            op=OP.subtract,
                        )
                    # bf16 2x min tree over 32 chunks
                    cur = trb[:].rearrange("p (c m) -> p c m", m=Mh)
                    for wnext in levels:
                        nxt = wp.tile([128, 32 * wnext], DT.bfloat16,
                                      tag=f"tree{wnext}", name=f"tr{wnext}")
                        nc.vector.tensor_tensor(
                            out=nxt[:].rearrange("p (c m) -> p c m", m=wnext),
                            in0=cur[:, :, 0:wnext],
                            in1=cur[:, :, wnext:2 * wnext],
                            op=OP.min,
                        )
                        cur = nxt[:].rearrange("p (c m) -> p c m", m=wnext)
                    c0 = s * CH + tp * 32
                    nc.vector.tensor_reduce(
                        out=minsq[:, c0:c0 + 32],
                        in_=cur,
                        axis=mybir.AxisListType.X,
                        op=OP.min,
                    )

            def emit_focal():
                nc.vector.tensor_scalar(
                    out=y[:], in0=tgt[:], scalar1=-2.0, scalar2=1.0,
                    op0=OP.mult, op1=OP.add,
                )
                nc.gpsimd.tensor_tensor(out=y[:], in0=y[:], in1=lgt[:], op=OP.mult)
                nc.scalar.activation(ce[:], y[:], F.Exp)
                nc.scalar.activation(ce[:], ce[:], F.Ln, bias=1.0)
                nc.scalar.activation(u2[:], y[:], F.Exp, scale=-1.0)
                nc.scalar.activation(u2[:], u2[:], F.Ln, bias=1.0)
                nc.scalar.activation(u2[:], u2[:], F.Exp, scale=-2.0)
                nc.gpsimd.tensor_tensor(out=fo[:], in0=ce[:], in1=u2[:], op=OP.mult)
                nc.vector.tensor_reduce(
                    out=sums[:, 0:1], in_=fo[:], axis=mybir.AxisListType.X,
                    op=OP.add,
                )

            def emit_epilogue(h):
                c0, c1 = h * (COLS // 2), (h + 1) * (COLS // 2)
                ms = minsq[:, c0:c1]
                nc.vector.tensor_scalar_max(out=ms, in0=ms, scalar1=1e-12)
                nc.scalar.activation(ms, ms, F.Ln)
                nc.scalar.activation(ms, ms, F.Exp, scale=0.5)    # sqrt
                nc.scalar.activation(ms, ms, F.Exp, scale=-10.0)  # w
                nc.gpsimd.tensor_tensor(out=y[:, c0:c1], in0=fo[:, c0:c1],
                                        in1=ms, op=OP.mult)
                nc.vector.tensor_reduce(
                    out=sums[:, 2 + h:3 + h], in_=y[:, c0:c1],
                    axis=mybir.AxisListType.X, op=OP.add,
                )

            emit_grid(0)
            emit_focal()
            for s in range(1, S):
                emit_grid(s)
                if s == S - 2:
                    emit_epilogue(0)   # samples 0..3 columns are final
            emit_epilogue(1)
            nc.sync.dma_start(out=out[:], in_=sums[:])

    split_waits(nc)
    return nc


def pack_inputs(inputs, targets, point_coords, corner_coords):
    """Host-side shard + layout packing. Returns (in_maps, Mk)."""
    x = np.asarray(inputs, np.float32)
    t = np.asarray(targets, np.float32)
    pc = np.asarray(point_coords, np.float32)
    cc = np.asarray(corner_coords, np.float32)

    pts = pc[..., :3]
    q = (pts * pts).sum(-1)
    feats = np.empty((B, K, N), np.float32)
    feats[:, 0] = pts[..., 0]
    feats[:, 1] = pts[..., 1]
    feats[:, 2] = pts[..., 2]
    feats[:, 3] = q
    feats[:, 4] = 1.0
    # [B, K, GRP, CPG, 128] -> [B, CPG, K, GRP, 128] -> [B, K*CPG, GRP*128]
    fg = feats.reshape(B, K, GRP, CPG, 128).transpose(0, 3, 1, 2, 4)
    lhsT = fg.reshape(B, K * CPG, N // CPG).astype(np.float16)

    # corners: compact valid to front, pad with PEN sentinels at origin
    valid = cc[..., 0] > -1.0
    nv = valid.sum(-1)
    maxv = int(nv.max()) if nv.max() > 0 else 1
    Mk = min(M, ((maxv + 31) // 32) * 32)
    Mh = Mk // 2
    cfeat = np.zeros((B, K, Mk), np.float32)
    cfeat[:, 4] = PEN
    for b in range(B):
        v = cc[b][valid[b]]
        n = v.shape[0]
        cfeat[b, 0, :n] = -2.0 * v[:, 0]
        cfeat[b, 1, :n] = -2.0 * v[:, 1]
        cfeat[b, 2, :n] = -2.0 * v[:, 2]
        cfeat[b, 3, :n] = 1.0
        cfeat[b, 4, :n] = (v * v).sum(-1)
    # pairwise: A features (even corners), E features (even - odd)
    fA = cfeat[:, :, 0::2]                       # [B, K, Mh]
    fE = fA - cfeat[:, :, 1::2]                  # [B, K, Mh]
    blk = np.concatenate([fA, fE], axis=2)       # [B, K, Mk]: [A | E]
    rhs = np.zeros((B, CPG, K * CPG, Mk), np.float32)
    for v in range(CPG):
        rhs[:, v, v * K:(v + 1) * K, :] = blk
    rhs = rhs.astype(np.float16)

    in_maps = []
    for c in range(NCORES):
        sl = slice(c * S, (c + 1) * S)
        lgp = x[sl].reshape(S, CH, 128).transpose(2, 0, 1).reshape(128, COLS).copy()
        tgp = t[sl].reshape(S, CH, 128).transpose(2, 0, 1).reshape(128, COLS).copy()
        rhp = rhs[sl].transpose(2, 0, 1, 3).reshape(K * CPG, S * CPG * Mk).copy()
        in_maps.append({
            "lhsT": np.ascontiguousarray(lhsT[sl]),
            "rhs": rhp,
            "lg": lgp,
            "tg": tgp,
        })
    return in_maps, Mk


def _finalize(results):
    s1 = 0.0
    s2 = 0.0
    for r in results:
        o = np.asarray(r["out"], np.float64)
        s1 += o[:, 0].sum()
        s2 += o[:, 2].sum() + o[:, 3].sum()
    bn = float(B * N)
    focal = s1 / bn
    distance = (s1 + 2.0 * s2) / bn
    total = focal + distance
    return (np.float32(total), np.float32(focal), np.float32(distance))


def kernel(inputs, targets, point_coords, corner_coords):
    in_maps, Mk = pack_inputs(inputs, targets, point_coords, corner_coords)
    if Mk not in _CACHE:
        _CACHE[Mk] = build_nc(Mk)
    nc = _CACHE[Mk]
    res = run_bass_kernel_spmd(nc, in_maps, core_ids=list(range(NCORES)))
    return _finalize(res.results)


if __name__ == "__main__":
    rng = np.random.default_rng(0)
    ins = {
        "inputs": rng.standard_normal((B, N), dtype=np.float32),
        "targets": (rng.random((B, N)) < 0.05).astype(np.float32),
        "point_coords": rng.random((B, N, 6), dtype=np.float32),
        "corner_coords": rng.random((B, 128, 3), dtype=np.float32),
    }
    print(kernel(**ins))
